# revision 56
# baseline (speedup 1.0000x reference)
"""CRF-RNN layer (nn_CrfRnnLayer) as a Trainium2 Bass kernel on 8 NeuronCores.

Math
----
The reference iterates, for q in R^{2xN} (N=3136 pixels, 2 classes):
    s         = softmax(q, axis=0)            (or s = unaries on iter 0)
    sp_out    = (s @ K_sp) / (K_sp @ 1)
    bl_out    = (s @ K_bl) / (K_bl @ 1)
    message   = sp_w @ sp_out + bl_w @ bl_out
    q         = unaries - compat @ message
Both rows of s sum to one (softmax; unaries too), and both kernel matrices
are symmetric, so the whole update collapses to a scalar recursion on
d = q[0] - q[1]:
    s0   = sigmoid(d)                        (s = [s0, 1-s0])
    v[i] = sum_j C[j,i] * s0[j]
    d    = U - v
with C = A*K_sp/nsp + B*K_bl/nbl (column-normalized), U = (1-2u) - G, and
A, B, G scalars derived from the 2x2 weight matrices.  The final output is
softmax(q)[1] = sigmoid(-d).

Device strategy (8 cores)
-------------------------
Column-shard C: core c owns columns i in [392c, 392(c+1)).  Each core:
  1. builds its [3136 x 392] block of exp(-0.5*sqdist) for both kernels
     on-chip via PE matmuls over augmented bf16 features (hi/lo splits
     keep the exponent exact to ~2e-3), then ScalarE Exp.  The spatial
     pass absorbs ln(4096*|A|/nsp) as a feature row so its Exp lands
     directly in the prescaled fp8 C tile; the bilateral pass is column-
     summed (ones-matmuls) for 4096*B/nbl, applied by DVE before a
     halved merge-add that unblocks the first matvec groups early;
  2. iterates: a u-seed matmul plus 14 fp8 DoubleRow matmuls accumulate
     -4096*d in PSUM (s stored [112, 2, 16] so contraction pairs j-tiles
     (g, g+14) with a 16B subtile step), ScalarE sigmoid reads the PSUM
     straight into an fp8 s row, and a 392B AllGather (Shared HBM
     output) redistributes it; warm dummy matmuls keep the PE clock up
     through the ~12us collective latency, and a startup throwaway
     collective on separate buffers absorbs the CC entry barrier.
Only the last iteration skips the collective and writes sigmoid(-d).
"""

import sys

for _p in ("/root/.axon_site/_ro/trn_rl_repo", "/opt/trn_rl_repo"):
    if _p not in sys.path:
        sys.path.append(_p)

import numpy as np

import concourse.bass as bass  # noqa: F401  (registers AP types)
import concourse.tile as tile
from concourse import bacc, mybir, bass_utils

F32 = mybir.dt.float32
BF16 = mybir.dt.bfloat16
AF = mybir.ActivationFunctionType
ALU = mybir.AluOpType

H = W = 56
N = H * W            # 3136 pixels
NC = 8               # cores
SHARD = N // NC      # 392 columns per core
P = 112              # j partition-tile height (112*28 == 3136)
T = 28               # number of j tiles
ITERS = 10
TH_ALPHA, TH_BETA, TH_GAMMA = 160.0, 3.0, 3.0

_BANK = 512          # one PSUM bank, in f32 elements
_GRP = 3             # exponent tiles batched per ScalarE Exp call
_WARM = 72           # HAM warm-keeping matmuls issued under each collective.
                     # 56 would end nearer the fast-mesh s-arrival, but every
                     # _WARM=56 run measured a degraded chip clock (+20% on
                     # all engines) while 72 mostly ran clean: the ~2us of
                     # dummy overrun buys sustained-activity clock residency
_CSC = 4096.0        # fp8 prescale on C (exact power of two)
_CPAD = 392          # fp8 cmat per-tile column pitch
_GH = T // 2         # DoubleRow groups per matvec (14)


def _build(a_val: float, b_val: float) -> "bacc.Bacc":
    nc = bacc.Bacc("TRN2", target_bir_lowering=False, debug=False,
                   num_devices=NC)

    F8 = mybir.dt.float8e4
    NA = T * 128     # a-feature tiles padded 112->128 cols: full-width
                     # stationaries enable FWL (LDWEIGHTS hides behind the
                     # previous matmul's stream)
    fbl_in = nc.dram_tensor("fbl", [128, NA], BF16, kind="ExternalInput").ap()
    fsp_in = nc.dram_tensor("fsp", [128, NA], BF16, kind="ExternalInput").ap()
    gbl_in = nc.dram_tensor("gbl", [128, SHARD], BF16,
                            kind="ExternalInput").ap()
    gsp_in = nc.dram_tensor("gsp", [128, SHARD], BF16,
                            kind="ExternalInput").ap()
    u_in = nc.dram_tensor("u", [2, SHARD], BF16, kind="ExternalInput").ap()
    one2_in = nc.dram_tensor("one2", [2, 1], BF16, kind="ExternalInput").ap()
    s0_in = nc.dram_tensor("s0", [P, 32], F8, kind="ExternalInput").ap()
    onec_in = nc.dram_tensor("onec", [P, 1], BF16, kind="ExternalInput").ap()
    oner_in = nc.dram_tensor("oner", [1, P], F32, kind="ExternalInput").ap()
    out = nc.dram_tensor("out", [1, SHARD], F32, kind="ExternalOutput").ap()
    sink = nc.dram_tensor("sink", [1, 1], F32, kind="ExternalOutput").ap()
    # collective buffers: Local input, Shared output (fast HBM-HBM path).
    # The throwaway warmup collective gets its own pair so iteration 0's
    # input DMA never waits on the entry-barrier-delayed warmup read.
    di_d = [nc.dram_tensor(f"di{k}", [SHARD], F8, kind="Internal").ap()
            for k in range(3)]
    do_d = [nc.dram_tensor(f"do{k}", [N], F8, kind="Internal",
                           addr_space="Shared").ap()
            for k in range(3)]

    groups = [list(range(g, min(g + _GRP, T))) for g in range(0, T, _GRP)]

    with tile.TileContext(nc) as tc:
        with (
            tc.tile_pool(name="const", bufs=1) as cpool,
            tc.tile_pool(name="emat", bufs=1) as epool,
            tc.tile_pool(name="row", bufs=2) as rpool,
            tc.tile_pool(name="sten", bufs=2) as spool,
            tc.tile_pool(name="dram", bufs=2, space="DRAM") as dpool,
        ):
            # throwaway AllGather input first: its trigger starts the
            # collectives entry barrier clock before the big feature DMAs
            # occupy the queue
            nc.sync.dma_start(
                di_d[2][:], s0_in[:].rearrange("p t -> (p t)")[0:SHARD])
            nc.gpsimd.collective_compute(
                "AllGather", ALU.bypass,
                replica_groups=[list(range(NC))],
                ins=[di_d[2][:].opt()], outs=[do_d[2][:].opt()],
            )

            # exponent-feature operands are zero-padded to 128 contraction
            # rows: a 4/5-row matmul doesn't register as PE activity, so the
            # HAM clock gate keeps the whole construction at 1.2 GHz.
            # Host sends the pad rows pre-zeroed (DMA engines are idle at
            # startup, the DVE memsets were on the critical path); the
            # bilateral operands ride first so the first exp matmul can
            # start before the spatial features land.
            # gbl (small, needed by every matmul) first; fbl arrives in
            # quarters so the first exp matmuls, which only read the
            # leading j-tiles, start ~2us before the full tensor lands
            # (the tile framework's AP-range deps make this safe)
            gbl_t = cpool.tile([128, SHARD], BF16, tag="gbl")
            nc.sync.dma_start(gbl_t[:], gbl_in[:])
            fbl_t = cpool.tile([128, NA], BF16, tag="fbl")
            for q in range(4):
                qs = slice(q * (NA // 4), (q + 1) * (NA // 4))
                nc.sync.dma_start(fbl_t[:, qs], fbl_in[:, qs])
            fsp_t = cpool.tile([128, NA], BF16, tag="fsp")
            nc.sync.dma_start(fsp_t[:], fsp_in[:])
            gsp_t = cpool.tile([128, SHARD], BF16, tag="gsp")
            nc.sync.dma_start(gsp_t[:], gsp_in[:])
            u_t = cpool.tile([2, SHARD], BF16, tag="u")
            nc.sync.dma_start(u_t[:], u_in[:])
            ones2 = cpool.tile([2, 1], BF16, tag="one2")
            nc.sync.dma_start(ones2[:], one2_in[:])
            s0_t = cpool.tile([P, 32], F8, tag="s0")
            nc.sync.dma_start(s0_t[:], s0_in[:])
            ones_col = cpool.tile([P, 1], BF16, tag="onec")
            nc.sync.dma_start(ones_col[:], onec_in[:])
            ones_row = cpool.tile([1, P], F32, tag="oner")
            nc.sync.dma_start(ones_row[:], oner_in[:])

            ebl = epool.tile([P, T * SHARD], BF16, tag="ebl")
            esp = epool.tile([P, T * SHARD], BF16, tag="esp")
            cmat = epool.tile([P, T * _CPAD], F8, tag="cmat")

            # ---- phase 1: exponent matmuls + exp + column sums ----
            # bilateral pass first: its serial tail (colsum -> recip ->
            # rb broadcast -> ebl*rb) overlaps the spatial pass, whose exp
            # (with ln(CSC*|A|/nsp) absorbed as two feature rows) lands
            # directly in the fp8 cmat.
            with (
                tc.tile_pool(name="psg", bufs=2, space="PSUM") as psg,
                tc.tile_pool(name="pss", bufs=1, space="PSUM") as pss,
            ):
                cs_bl = pss.tile([1, SHARD], F32, tag="cs_bl")

                def exp_pass(fa_t, fb_t, dst3, cs, scale, post=None):
                    for grp in groups:
                        pg = psg.tile([128, _GRP * _BANK], F32, tag="grp")
                        for k, t in enumerate(grp):
                            nc.tensor.matmul(
                                pg[:, k * _BANK : k * _BANK + SHARD],
                                fa_t[:, t * 128 : (t + 1) * 128],
                                fb_t[:],
                                start=True, stop=True,
                                skip_group_check=True,
                            )
                        ln = len(grp)
                        src = pg[:].rearrange("p (k f) -> p k f", f=_BANK)[
                            0:P, 0:ln, 0:SHARD]
                        nc.scalar.activation(dst3[:, grp[0] : grp[0] + ln, :],
                                             src, AF.Exp, scale=scale)
                        for t in (grp if cs is not None else []):
                            nc.tensor.matmul(
                                cs[:],
                                ones_col[:],
                                ebl[:, t * SHARD : (t + 1) * SHARD],
                                start=(t == 0), stop=(t == T - 1),
                                skip_group_check=True,
                            )
                        if post is not None:
                            post(grp[0], ln)

                b3 = ebl[:].rearrange("p (k f) -> p k f", f=SHARD)
                e3 = esp[:].rearrange("p (k f) -> p k f", f=SHARD)
                c3 = cmat[:].rearrange("p (k f) -> p k f", f=_CPAD
                                       )[:, :, 0:SHARD]
                exp_pass(fbl_t, gbl_t, b3, cs_bl, -1.0 / 6.0)

                # rb = CSC*B/nbl, broadcast down the partitions on the PE
                rb_row = cpool.tile([1, SHARD], F32, tag="rb")
                nc.vector.reciprocal(rb_row[:], cs_bl[:])
                rb_bc = pss.tile([P, SHARD], F32, tag="rbbc")
                nc.tensor.matmul(rb_bc[:], ones_row[:], rb_row[:],
                                 start=True, stop=True, skip_group_check=True)
                rb_sb = cpool.tile([P, SHARD], BF16, tag="rbsb")
                nc.vector.tensor_scalar_mul(rb_sb[:], rb_bc[:],
                                            float(b_val * _CSC))

                # ebl *= rb (one DVE pass) runs under the spatial Exp pass;
                # each spatial group's fp8 merge (all-bf16 inputs) then
                # chases its activation, so only the last ~0.6us of DVE work
                # trails the final Exp before the first matvec can start
                nc.vector.tensor_mul(
                    b3, b3,
                    rb_sb[:].rearrange("p (o f) -> p o f", o=1
                                       ).broadcast_to([P, T, SHARD]))

                def merge_grp(g0, ln):
                    cg = c3[:, g0 : g0 + ln, :]
                    eg = e3[:, g0 : g0 + ln, :]
                    bg = b3[:, g0 : g0 + ln, :]
                    if a_val >= 0.0:
                        nc.vector.tensor_add(cg, eg, bg)
                    else:
                        nc.vector.tensor_sub(cg, bg, eg)

                exp_pass(fsp_t, gsp_t, e3, None, 1.0 / 9.0, post=merge_grp)

                # swap the sigmoid ACT table in behind the first matvec;
                # reading the last Exp group's output pins this after the
                # Exp pass (a dep-free pre-warm gets hoisted ahead of it)
                pre_sg = cpool.tile([1, 1], F32, tag="presg")
                nc.scalar.activation(
                    pre_sg[:], esp[0:1, T * SHARD - 1 : T * SHARD],
                    AF.Sigmoid)

            # ---- phase 3: CRF mean-field iterations ----
            # psum accumulates -CSC*d = (-CSC*u seed) + sum_t CSC*C^T s.
            # fp8 DoubleRow matvec: group g contracts j-tiles (g, g+_GH);
            # s is stored [P, 2, 16] (14 used + 2 pad, 16B subtile step).
            with (
                tc.tile_pool(name="psv", bufs=2, space="PSUM") as psv,
                tc.tile_pool(name="psd", bufs=1, space="PSUM") as psd,
            ):
                dummy = psd.tile([1, SHARD], F32, tag="dummy")
                cm3 = cmat[:].rearrange("p (e g) -> p e g", e=2,
                                        g=_GH * _CPAD)
                DR = mybir.MatmulPerfMode.DoubleRow

                def seed(vt):
                    nc.tensor.matmul(
                        vt[:], ones2[:], u_t[:],
                        start=True, stop=False, skip_group_check=True,
                    )

                s_cur = s0_t
                v = psv.tile([1, SHARD], F32, tag="v")
                seed(v)
                for it in range(ITERS):
                    s3 = s_cur[:].rearrange("p (e g) -> p e g", e=2, g=16)
                    for g in range(_GH):
                        nc.tensor.matmul(
                            v[:],
                            s3[:, :, g : g + 1],
                            cm3[:, :, g * _CPAD : g * _CPAD + SHARD],
                            start=False, stop=(g == _GH - 1),
                            perf_mode=DR,
                            skip_group_check=True,
                        )
                    if it < ITERS - 1:
                        s_row = rpool.tile([1, SHARD], F8, tag="srow")
                        nc.scalar.activation(s_row[:], v[:], AF.Sigmoid,
                                             scale=-1.0 / _CSC)
                        di, do = di_d[it % 2], do_d[it % 2]
                        # scalar-issued DMA: no cross-engine semaphore hop
                        # between the sigmoid and the collective input
                        nc.scalar.dma_start(
                            di[:].rearrange("(a b) -> a b", a=1), s_row[:])
                        nc.gpsimd.collective_compute(
                            "AllGather", ALU.bypass,
                            replica_groups=[list(range(NC))],
                            ins=[di[:].opt()], outs=[do[:].opt()],
                        )
                        # keep the PE HAM-warm through the collective gap.
                        # The first ("linker") matmul reads s_row, so the
                        # whole WAW-chained dummy block is ordered after the
                        # sigmoid — it cannot interleave into the matvec
                        # accumulation and delay v's ready semaphore.  The
                        # next iteration's u-seed hides under the collective
                        # too, right behind the linker.
                        nc.tensor.matmul(
                            dummy[:], s_row[0:1, 0:1], s_row[:],
                            start=True, stop=True, skip_group_check=True,
                        )
                        v = psv.tile([1, SHARD], F32, tag="v")
                        seed(v)
                        # the last gap feeds the final matvec directly:
                        # undershoot rather than risk dummies delaying it
                        nw = _WARM if it < ITERS - 2 else _WARM - 10
                        for w in range(nw):
                            c0 = (w % T) * _CPAD
                            nc.tensor.matmul(
                                dummy[:],
                                s_cur[:, (w % 28) : (w % 28) + 1],
                                cmat[:, c0 : c0 + SHARD],
                                start=True, stop=True,
                                skip_group_check=True,
                            )
                        s_nxt = spool.tile([P, 32], F8, tag="s")
                        nc.sync.dma_start(
                            s_nxt[:].rearrange("p (e g) -> p e g",
                                               e=2, g=16)[:, :, 0:_GH],
                            do[:].rearrange("(p e g) -> p e g", e=2, g=_GH))
                        s_cur = s_nxt
                    else:
                        o_row = rpool.tile([1, SHARD], F32, tag="orow")
                        nc.scalar.activation(o_row[:], v[:], AF.Sigmoid,
                                             scale=1.0 / _CSC)
                        nc.scalar.dma_start(out[:], o_row[:])
                sink_row = rpool.tile([1, 1], F32, tag="sink")
                nc.vector.tensor_copy(sink_row[:], dummy[0:1, 0:1])
                nc.sync.dma_start(sink[:], sink_row[:])

    nc.compile()
    return nc


def _host_prep(inputs, spatial_ker_weights, bilateral_ker_weights,
               compatibility_matrix):
    unary = np.asarray(inputs[0], dtype=np.float64)
    gray = np.asarray(inputs[1], dtype=np.float64)
    sp_w = np.asarray(spatial_ker_weights, dtype=np.float64)
    bl_w = np.asarray(bilateral_ker_weights, dtype=np.float64)
    compat = np.asarray(compatibility_matrix, dtype=np.float64)

    dsp = sp_w[:, 0] - sp_w[:, 1]
    dbl = bl_w[:, 0] - bl_w[:, 1]
    c0 = sp_w[:, 1] + bl_w[:, 1]
    dc = compat[0, :] - compat[1, :]
    a_val = float(dc @ dsp)
    b_val = float(dc @ dbl)
    g_val = float(dc @ c0)

    ys, xs = np.meshgrid(np.arange(H, dtype=np.float64),
                         np.arange(W, dtype=np.float64), indexing="ij")
    x = xs.ravel()
    y = ys.ravel()
    gf = gray.ravel() * 255.0

    import ml_dtypes
    _mld = ml_dtypes
    one = np.ones(N, dtype=np.float64)

    def bf(v):
        return np.asarray(v, dtype=_mld.bfloat16).astype(np.float64)

    def split3(v):
        a = bf(v)
        b = bf(v - a)
        c = bf(v - a - b)
        return a, b, c

    # spatial norm is a Kronecker product: nsp[(y,x)] = ry[y]*rx[x]
    idx = np.arange(H, dtype=np.float64)
    g1d = np.exp(-0.5 * ((idx[None, :] - idx[:, None]) / TH_GAMMA) ** 2)
    r1d = g1d.sum(axis=1)
    nsp = (r1d[y.astype(int)] * r1d[x.astype(int)])

    # spatial exponent in bf16-exact integer arithmetic, scaled by 1/9 at
    # the Exp activation; the fp8 prescale and the column norm are folded
    # in as a 9*ln(CSC*|A|/nsp) hi/lo feature pair:
    # presc = xj*xi + yj*yi - (x^2+y^2)/2 terms + ln rows
    ssp_i = 0.5 * (x * x + y * y)                 # multiples of 0.5
    sp_hi = bf(-ssp_i)
    sp_lo = -ssp_i - sp_hi                        # exact in bf16
    lr = 9.0 * np.log(np.maximum(4096.0 * abs(a_val) / nsp, 1e-280))
    lr = np.maximum(lr, -2000.0)
    lr_hi = bf(lr)
    lr_lo = lr - lr_hi

    asp_g = np.stack([x, y, one, one, sp_hi, sp_lo, one, one], axis=0)
    bsp_g = np.stack([x, y, sp_hi, sp_lo, one, one, lr_hi, lr_lo], axis=0)

    # bilateral exponent on bf16 features: presc = ssq_i + ssq_j
    # - 2*(xp_i xp_j + yp_i yp_j + g_i g_j), scaled by -1/6 at the Exp.
    # g and ssq are 3-way bf16 splits so every product is exact in f32;
    # validated max exponent error ~2e-3.
    s3f = np.sqrt(3.0)
    xp = bf(s3f * x / TH_ALPHA)
    yp = bf(s3f * y / TH_ALPHA)
    g1, g2, g3 = split3(gf)
    gs = g1 + g2 + g3
    ssq = xp * xp + yp * yp + gs * gs
    s1, s2, sr = split3(ssq)

    abl_g = np.stack([xp, yp, g1, g1, g2, g1, g3, g2,
                      s1, s2, sr, one, one, one], axis=0)
    bbl_g = np.stack([-2 * xp, -2 * yp, -2 * g1, -2 * g2, -2 * g1,
                      -2 * g3, -2 * g1, -2 * g2,
                      one, one, one, s1, s2, sr], axis=0)

    # device j-tiling: lhsT column t*P + p  <->  global j = p*T + t
    j_order = np.arange(N).reshape(P, T).T.ravel()

    def pad128(m, cols):
        out = np.zeros((128, cols), dtype=_mld.bfloat16)
        out[: m.shape[0]] = m.astype(_mld.bfloat16)
        return out

    def pad_tiles(m):
        # [nf, N] j-ordered features -> [128, T*128]: each 112-col j-tile
        # padded to 128 cols so the exp matmuls get full-width stationaries
        out = np.zeros((128, T * 128), dtype=_mld.bfloat16)
        mj = m.astype(_mld.bfloat16)
        for t in range(T):
            out[: m.shape[0], t * 128 : t * 128 + P] = \
                mj[:, t * P : (t + 1) * P]
        return out

    fbl_d = pad_tiles(abl_g[:, j_order])
    fsp_d = pad_tiles(asp_g[:, j_order])

    def hilo(v):
        hi = np.asarray(v, dtype=_mld.bfloat16).astype(np.float64)
        lo = np.asarray(v - hi, dtype=_mld.bfloat16)
        return np.stack([hi.astype(_mld.bfloat16), lo], axis=0)

    u_flat = unary.ravel()
    u_full = hilo(-4096.0 * ((1.0 - 2.0 * u_flat) - g_val))    # [2, N] bf16
    s0_pt = (1.0 - u_flat).reshape(P, T)
    s0_dev = np.zeros((P, 32), dtype=ml_dtypes.float8_e4m3fn)
    s0_dev[:, 0:14] = s0_pt[:, 0:14].astype(ml_dtypes.float8_e4m3fn)
    s0_dev[:, 16:30] = s0_pt[:, 14:28].astype(ml_dtypes.float8_e4m3fn)

    in_maps = []
    for c in range(NC):
        sl = slice(c * SHARD, (c + 1) * SHARD)
        in_maps.append({
            "fbl": fbl_d,
            "fsp": fsp_d,
            "gbl": pad128(bbl_g[:, sl], SHARD),
            "gsp": pad128(bsp_g[:, sl], SHARD),
            "u": np.ascontiguousarray(u_full[:, sl]),
            "s0": s0_dev,
            "onec": np.ones((P, 1), dtype=ml_dtypes.bfloat16),
            "one2": np.ones((2, 1), dtype=ml_dtypes.bfloat16),
            "oner": np.ones((1, P), dtype=np.float32),
        })
    return a_val, b_val, in_maps


_CACHE = {}


def kernel(inputs, spatial_ker_weights, bilateral_ker_weights,
           compatibility_matrix, _want_results=False):
    a_val, b_val, in_maps = _host_prep(
        inputs, spatial_ker_weights, bilateral_ker_weights,
        compatibility_matrix)

    key = (a_val, b_val)
    if key not in _CACHE:
        _CACHE[key] = _build(a_val, b_val)
    nc = _CACHE[key]

    res = bass_utils.run_bass_kernel_spmd(nc, in_maps, list(range(NC)))
    prob = np.concatenate([res.results[c]["out"][0] for c in range(NC)])
    out = prob.reshape(1, H, W).astype(np.float32)
    if _want_results:
        return out, nc, in_maps
    return out


if __name__ == "__main__":
    rng = np.random.default_rng(0)
    demo = {
        "inputs": rng.random((2, H, W)).astype(np.float32),
        "spatial_ker_weights":
            (rng.random((2, 2)).astype(np.float32) - 0.5) * 0.1,
        "bilateral_ker_weights":
            (rng.random((2, 2)).astype(np.float32) - 0.5) * 0.1,
        "compatibility_matrix":
            (rng.random((2, 2)).astype(np.float32) - 0.5) * 0.1,
    }
    print(kernel(**demo).shape)



# revision 57
# speedup vs baseline: 1.0617x; 1.0617x over previous
"""CRF-RNN layer (nn_CrfRnnLayer) as a Trainium2 Bass kernel on 8 NeuronCores.

Math
----
The reference iterates, for q in R^{2xN} (N=3136 pixels, 2 classes):
    s         = softmax(q, axis=0)            (or s = unaries on iter 0)
    sp_out    = (s @ K_sp) / (K_sp @ 1)
    bl_out    = (s @ K_bl) / (K_bl @ 1)
    message   = sp_w @ sp_out + bl_w @ bl_out
    q         = unaries - compat @ message
Both rows of s sum to one (softmax; unaries too), and both kernel matrices
are symmetric, so the whole update collapses to a scalar recursion on
d = q[0] - q[1]:
    s0   = sigmoid(d)                        (s = [s0, 1-s0])
    v[i] = sum_j C[j,i] * s0[j]
    d    = U - v
with C = A*K_sp/nsp + B*K_bl/nbl (column-normalized), U = (1-2u) - G, and
A, B, G scalars derived from the 2x2 weight matrices.  The final output is
softmax(q)[1] = sigmoid(-d).

Device strategy (8 cores)
-------------------------
Column-shard C: core c owns columns i in [392c, 392(c+1)).  Each core:
  1. builds its [3136 x 392] block of exp(-0.5*sqdist) for both kernels
     on-chip via PE matmuls over augmented bf16 features (hi/lo splits
     keep the exponent exact to ~2e-3), then ScalarE Exp.  The spatial
     pass absorbs ln(4096*|A|/nsp) as a feature row so its Exp lands
     directly in the prescaled fp8 C tile; the bilateral pass is column-
     summed (ones-matmuls) for 4096*B/nbl, applied by DVE before a
     halved merge-add that unblocks the first matvec groups early;
  2. iterates: a u-seed matmul plus 14 fp8 DoubleRow matmuls accumulate
     -4096*d in PSUM (s stored [112, 2, 16] so contraction pairs j-tiles
     (g, g+14) with a 16B subtile step), ScalarE sigmoid reads the PSUM
     straight into an fp8 s row, and a 392B AllGather (Shared HBM
     output) redistributes it; warm dummy matmuls keep the PE clock up
     through the ~12us collective latency, and a startup throwaway
     collective on separate buffers absorbs the CC entry barrier.
Only the last iteration skips the collective and writes sigmoid(-d).
"""

import sys

for _p in ("/root/.axon_site/_ro/trn_rl_repo", "/opt/trn_rl_repo"):
    if _p not in sys.path:
        sys.path.append(_p)

import numpy as np

import concourse.bass as bass  # noqa: F401  (registers AP types)
import concourse.tile as tile
from concourse import bacc, mybir, bass_utils

F32 = mybir.dt.float32
BF16 = mybir.dt.bfloat16
AF = mybir.ActivationFunctionType
ALU = mybir.AluOpType

H = W = 56
N = H * W            # 3136 pixels
NC = 8               # cores
SHARD = N // NC      # 392 columns per core
P = 112              # j partition-tile height (112*28 == 3136)
T = 28               # number of j tiles
ITERS = 10
TH_ALPHA, TH_BETA, TH_GAMMA = 160.0, 3.0, 3.0

_BANK = 512          # one PSUM bank, in f32 elements
_GRP = 3             # exponent tiles batched per ScalarE Exp call
_WARM = 56           # HAM warm-keeping matmuls issued under each collective.
                     # Collective latency is clock-independent but dummy
                     # duration is not: 56 ends at s-arrival on a full-clock
                     # run and overruns ~1us degraded, beating 72 in both
                     # states (72 was dummy-bound at 15.76us/18.92us periods)
_CSC = 4096.0        # fp8 prescale on C (exact power of two)
_CPAD = 392          # fp8 cmat per-tile column pitch
_GH = T // 2         # DoubleRow groups per matvec (14)


def _build(a_val: float, b_val: float) -> "bacc.Bacc":
    nc = bacc.Bacc("TRN2", target_bir_lowering=False, debug=False,
                   num_devices=NC)

    F8 = mybir.dt.float8e4
    NA = T * 128     # a-feature tiles padded 112->128 cols: full-width
                     # stationaries enable FWL (LDWEIGHTS hides behind the
                     # previous matmul's stream)
    fbl_in = nc.dram_tensor("fbl", [128, NA], BF16, kind="ExternalInput").ap()
    fsp_in = nc.dram_tensor("fsp", [128, NA], BF16, kind="ExternalInput").ap()
    gbl_in = nc.dram_tensor("gbl", [128, SHARD], BF16,
                            kind="ExternalInput").ap()
    gsp_in = nc.dram_tensor("gsp", [128, SHARD], BF16,
                            kind="ExternalInput").ap()
    u_in = nc.dram_tensor("u", [2, SHARD], BF16, kind="ExternalInput").ap()
    one2_in = nc.dram_tensor("one2", [2, 1], BF16, kind="ExternalInput").ap()
    s0_in = nc.dram_tensor("s0", [P, 32], F8, kind="ExternalInput").ap()
    onec_in = nc.dram_tensor("onec", [P, 1], BF16, kind="ExternalInput").ap()
    oner_in = nc.dram_tensor("oner", [1, P], F32, kind="ExternalInput").ap()
    out = nc.dram_tensor("out", [1, SHARD], F32, kind="ExternalOutput").ap()
    sink = nc.dram_tensor("sink", [1, 1], F32, kind="ExternalOutput").ap()
    # collective buffers: Local input, Shared output (fast HBM-HBM path).
    # The throwaway warmup collective gets its own pair so iteration 0's
    # input DMA never waits on the entry-barrier-delayed warmup read.
    di_d = [nc.dram_tensor(f"di{k}", [SHARD], F8, kind="Internal").ap()
            for k in range(3)]
    do_d = [nc.dram_tensor(f"do{k}", [N], F8, kind="Internal",
                           addr_space="Shared").ap()
            for k in range(3)]

    groups = [list(range(g, min(g + _GRP, T))) for g in range(0, T, _GRP)]

    with tile.TileContext(nc) as tc:
        with (
            tc.tile_pool(name="const", bufs=1) as cpool,
            tc.tile_pool(name="emat", bufs=1) as epool,
            tc.tile_pool(name="row", bufs=2) as rpool,
            tc.tile_pool(name="sten", bufs=2) as spool,
            tc.tile_pool(name="dram", bufs=2, space="DRAM") as dpool,
        ):
            # throwaway AllGather input first: its trigger starts the
            # collectives entry barrier clock before the big feature DMAs
            # occupy the queue
            nc.sync.dma_start(
                di_d[2][:], s0_in[:].rearrange("p t -> (p t)")[0:SHARD])
            nc.gpsimd.collective_compute(
                "AllGather", ALU.bypass,
                replica_groups=[list(range(NC))],
                ins=[di_d[2][:].opt()], outs=[do_d[2][:].opt()],
            )

            # exponent-feature operands are zero-padded to 128 contraction
            # rows: a 4/5-row matmul doesn't register as PE activity, so the
            # HAM clock gate keeps the whole construction at 1.2 GHz.
            # Host sends the pad rows pre-zeroed (DMA engines are idle at
            # startup, the DVE memsets were on the critical path); the
            # bilateral operands ride first so the first exp matmul can
            # start before the spatial features land.
            # gbl (small, needed by every matmul) first; fbl arrives in
            # quarters so the first exp matmuls, which only read the
            # leading j-tiles, start ~2us before the full tensor lands
            # (the tile framework's AP-range deps make this safe)
            gbl_t = cpool.tile([128, SHARD], BF16, tag="gbl")
            nc.sync.dma_start(gbl_t[:], gbl_in[:])
            fbl_t = cpool.tile([128, NA], BF16, tag="fbl")
            for q in range(4):
                qs = slice(q * (NA // 4), (q + 1) * (NA // 4))
                nc.sync.dma_start(fbl_t[:, qs], fbl_in[:, qs])
            fsp_t = cpool.tile([128, NA], BF16, tag="fsp")
            nc.sync.dma_start(fsp_t[:], fsp_in[:])
            gsp_t = cpool.tile([128, SHARD], BF16, tag="gsp")
            nc.sync.dma_start(gsp_t[:], gsp_in[:])
            u_t = cpool.tile([2, SHARD], BF16, tag="u")
            nc.sync.dma_start(u_t[:], u_in[:])
            ones2 = cpool.tile([2, 1], BF16, tag="one2")
            nc.sync.dma_start(ones2[:], one2_in[:])
            s0_t = cpool.tile([P, 32], F8, tag="s0")
            nc.sync.dma_start(s0_t[:], s0_in[:])
            ones_col = cpool.tile([P, 1], BF16, tag="onec")
            nc.sync.dma_start(ones_col[:], onec_in[:])
            ones_row = cpool.tile([1, P], F32, tag="oner")
            nc.sync.dma_start(ones_row[:], oner_in[:])

            ebl = epool.tile([P, T * SHARD], BF16, tag="ebl")
            esp = epool.tile([P, T * SHARD], BF16, tag="esp")
            cmat = epool.tile([P, T * _CPAD], F8, tag="cmat")

            # ---- phase 1: exponent matmuls + exp + column sums ----
            # bilateral pass first: its serial tail (colsum -> recip ->
            # rb broadcast -> ebl*rb) overlaps the spatial pass, whose exp
            # (with ln(CSC*|A|/nsp) absorbed as two feature rows) lands
            # directly in the fp8 cmat.
            with (
                tc.tile_pool(name="psg", bufs=2, space="PSUM") as psg,
                tc.tile_pool(name="pss", bufs=1, space="PSUM") as pss,
            ):
                cs_bl = pss.tile([1, SHARD], F32, tag="cs_bl")

                def exp_pass(fa_t, fb_t, dst3, cs, scale, post=None):
                    for grp in groups:
                        pg = psg.tile([128, _GRP * _BANK], F32, tag="grp")
                        for k, t in enumerate(grp):
                            nc.tensor.matmul(
                                pg[:, k * _BANK : k * _BANK + SHARD],
                                fa_t[:, t * 128 : (t + 1) * 128],
                                fb_t[:],
                                start=True, stop=True,
                                skip_group_check=True,
                            )
                        ln = len(grp)
                        src = pg[:].rearrange("p (k f) -> p k f", f=_BANK)[
                            0:P, 0:ln, 0:SHARD]
                        nc.scalar.activation(dst3[:, grp[0] : grp[0] + ln, :],
                                             src, AF.Exp, scale=scale)
                        for t in (grp if cs is not None else []):
                            nc.tensor.matmul(
                                cs[:],
                                ones_col[:],
                                ebl[:, t * SHARD : (t + 1) * SHARD],
                                start=(t == 0), stop=(t == T - 1),
                                skip_group_check=True,
                            )
                        if post is not None:
                            post(grp[0], ln)

                b3 = ebl[:].rearrange("p (k f) -> p k f", f=SHARD)
                e3 = esp[:].rearrange("p (k f) -> p k f", f=SHARD)
                c3 = cmat[:].rearrange("p (k f) -> p k f", f=_CPAD
                                       )[:, :, 0:SHARD]
                exp_pass(fbl_t, gbl_t, b3, cs_bl, -1.0 / 6.0)

                # rb = CSC*B/nbl, broadcast down the partitions on the PE
                rb_row = cpool.tile([1, SHARD], F32, tag="rb")
                nc.vector.reciprocal(rb_row[:], cs_bl[:])
                rb_bc = pss.tile([P, SHARD], F32, tag="rbbc")
                nc.tensor.matmul(rb_bc[:], ones_row[:], rb_row[:],
                                 start=True, stop=True, skip_group_check=True)
                rb_sb = cpool.tile([P, SHARD], BF16, tag="rbsb")
                nc.vector.tensor_scalar_mul(rb_sb[:], rb_bc[:],
                                            float(b_val * _CSC))

                # ebl *= rb (one DVE pass) runs under the spatial Exp pass;
                # each spatial group's fp8 merge (all-bf16 inputs) then
                # chases its activation, so only the last ~0.6us of DVE work
                # trails the final Exp before the first matvec can start
                nc.vector.tensor_mul(
                    b3, b3,
                    rb_sb[:].rearrange("p (o f) -> p o f", o=1
                                       ).broadcast_to([P, T, SHARD]))

                def merge_grp(g0, ln):
                    cg = c3[:, g0 : g0 + ln, :]
                    eg = e3[:, g0 : g0 + ln, :]
                    bg = b3[:, g0 : g0 + ln, :]
                    if a_val >= 0.0:
                        nc.vector.tensor_add(cg, eg, bg)
                    else:
                        nc.vector.tensor_sub(cg, bg, eg)

                exp_pass(fsp_t, gsp_t, e3, None, 1.0 / 9.0, post=merge_grp)

                # swap the sigmoid ACT table in behind the first matvec;
                # reading the last Exp group's output pins this after the
                # Exp pass (a dep-free pre-warm gets hoisted ahead of it)
                pre_sg = cpool.tile([1, 1], F32, tag="presg")
                nc.scalar.activation(
                    pre_sg[:], esp[0:1, T * SHARD - 1 : T * SHARD],
                    AF.Sigmoid)

            # ---- phase 3: CRF mean-field iterations ----
            # psum accumulates -CSC*d = (-CSC*u seed) + sum_t CSC*C^T s.
            # fp8 DoubleRow matvec: group g contracts j-tiles (g, g+_GH);
            # s is stored [P, 2, 16] (14 used + 2 pad, 16B subtile step).
            with (
                tc.tile_pool(name="psv", bufs=2, space="PSUM") as psv,
                tc.tile_pool(name="psd", bufs=1, space="PSUM") as psd,
            ):
                dummy = psd.tile([1, SHARD], F32, tag="dummy")
                cm3 = cmat[:].rearrange("p (e g) -> p e g", e=2,
                                        g=_GH * _CPAD)
                DR = mybir.MatmulPerfMode.DoubleRow

                def seed(vt):
                    nc.tensor.matmul(
                        vt[:], ones2[:], u_t[:],
                        start=True, stop=False, skip_group_check=True,
                    )

                s_cur = s0_t
                v = psv.tile([1, SHARD], F32, tag="v")
                seed(v)
                for it in range(ITERS):
                    s3 = s_cur[:].rearrange("p (e g) -> p e g", e=2, g=16)
                    for g in range(_GH):
                        nc.tensor.matmul(
                            v[:],
                            s3[:, :, g : g + 1],
                            cm3[:, :, g * _CPAD : g * _CPAD + SHARD],
                            start=False, stop=(g == _GH - 1),
                            perf_mode=DR,
                            skip_group_check=True,
                        )
                    if it < ITERS - 1:
                        s_row = rpool.tile([1, SHARD], F8, tag="srow")
                        nc.scalar.activation(s_row[:], v[:], AF.Sigmoid,
                                             scale=-1.0 / _CSC)
                        di, do = di_d[it % 2], do_d[it % 2]
                        # scalar-issued DMA: no cross-engine semaphore hop
                        # between the sigmoid and the collective input
                        nc.scalar.dma_start(
                            di[:].rearrange("(a b) -> a b", a=1), s_row[:])
                        nc.gpsimd.collective_compute(
                            "AllGather", ALU.bypass,
                            replica_groups=[list(range(NC))],
                            ins=[di[:].opt()], outs=[do[:].opt()],
                        )
                        # keep the PE HAM-warm through the collective gap.
                        # The first ("linker") matmul reads s_row, so the
                        # whole WAW-chained dummy block is ordered after the
                        # sigmoid — it cannot interleave into the matvec
                        # accumulation and delay v's ready semaphore.  The
                        # next iteration's u-seed hides under the collective
                        # too, right behind the linker.
                        nc.tensor.matmul(
                            dummy[:], s_row[0:1, 0:1], s_row[:],
                            start=True, stop=True, skip_group_check=True,
                        )
                        v = psv.tile([1, SHARD], F32, tag="v")
                        seed(v)
                        # the last gap feeds the final matvec directly:
                        # undershoot rather than risk dummies delaying it
                        nw = _WARM if it < ITERS - 2 else _WARM - 10
                        for w in range(nw):
                            c0 = (w % T) * _CPAD
                            nc.tensor.matmul(
                                dummy[:],
                                s_cur[:, (w % 28) : (w % 28) + 1],
                                cmat[:, c0 : c0 + SHARD],
                                start=True, stop=True,
                                skip_group_check=True,
                            )
                        s_nxt = spool.tile([P, 32], F8, tag="s")
                        nc.sync.dma_start(
                            s_nxt[:].rearrange("p (e g) -> p e g",
                                               e=2, g=16)[:, :, 0:_GH],
                            do[:].rearrange("(p e g) -> p e g", e=2, g=_GH))
                        s_cur = s_nxt
                    else:
                        o_row = rpool.tile([1, SHARD], F32, tag="orow")
                        nc.scalar.activation(o_row[:], v[:], AF.Sigmoid,
                                             scale=1.0 / _CSC)
                        nc.scalar.dma_start(out[:], o_row[:])
                sink_row = rpool.tile([1, 1], F32, tag="sink")
                nc.vector.tensor_copy(sink_row[:], dummy[0:1, 0:1])
                nc.sync.dma_start(sink[:], sink_row[:])

    nc.compile()
    return nc


def _host_prep(inputs, spatial_ker_weights, bilateral_ker_weights,
               compatibility_matrix):
    unary = np.asarray(inputs[0], dtype=np.float64)
    gray = np.asarray(inputs[1], dtype=np.float64)
    sp_w = np.asarray(spatial_ker_weights, dtype=np.float64)
    bl_w = np.asarray(bilateral_ker_weights, dtype=np.float64)
    compat = np.asarray(compatibility_matrix, dtype=np.float64)

    dsp = sp_w[:, 0] - sp_w[:, 1]
    dbl = bl_w[:, 0] - bl_w[:, 1]
    c0 = sp_w[:, 1] + bl_w[:, 1]
    dc = compat[0, :] - compat[1, :]
    a_val = float(dc @ dsp)
    b_val = float(dc @ dbl)
    g_val = float(dc @ c0)

    ys, xs = np.meshgrid(np.arange(H, dtype=np.float64),
                         np.arange(W, dtype=np.float64), indexing="ij")
    x = xs.ravel()
    y = ys.ravel()
    gf = gray.ravel() * 255.0

    import ml_dtypes
    _mld = ml_dtypes
    one = np.ones(N, dtype=np.float64)

    def bf(v):
        return np.asarray(v, dtype=_mld.bfloat16).astype(np.float64)

    def split3(v):
        a = bf(v)
        b = bf(v - a)
        c = bf(v - a - b)
        return a, b, c

    # spatial norm is a Kronecker product: nsp[(y,x)] = ry[y]*rx[x]
    idx = np.arange(H, dtype=np.float64)
    g1d = np.exp(-0.5 * ((idx[None, :] - idx[:, None]) / TH_GAMMA) ** 2)
    r1d = g1d.sum(axis=1)
    nsp = (r1d[y.astype(int)] * r1d[x.astype(int)])

    # spatial exponent in bf16-exact integer arithmetic, scaled by 1/9 at
    # the Exp activation; the fp8 prescale and the column norm are folded
    # in as a 9*ln(CSC*|A|/nsp) hi/lo feature pair:
    # presc = xj*xi + yj*yi - (x^2+y^2)/2 terms + ln rows
    ssp_i = 0.5 * (x * x + y * y)                 # multiples of 0.5
    sp_hi = bf(-ssp_i)
    sp_lo = -ssp_i - sp_hi                        # exact in bf16
    lr = 9.0 * np.log(np.maximum(4096.0 * abs(a_val) / nsp, 1e-280))
    lr = np.maximum(lr, -2000.0)
    lr_hi = bf(lr)
    lr_lo = lr - lr_hi

    asp_g = np.stack([x, y, one, one, sp_hi, sp_lo, one, one], axis=0)
    bsp_g = np.stack([x, y, sp_hi, sp_lo, one, one, lr_hi, lr_lo], axis=0)

    # bilateral exponent on bf16 features: presc = ssq_i + ssq_j
    # - 2*(xp_i xp_j + yp_i yp_j + g_i g_j), scaled by -1/6 at the Exp.
    # g and ssq are 3-way bf16 splits so every product is exact in f32;
    # validated max exponent error ~2e-3.
    s3f = np.sqrt(3.0)
    xp = bf(s3f * x / TH_ALPHA)
    yp = bf(s3f * y / TH_ALPHA)
    g1, g2, g3 = split3(gf)
    gs = g1 + g2 + g3
    ssq = xp * xp + yp * yp + gs * gs
    s1, s2, sr = split3(ssq)

    abl_g = np.stack([xp, yp, g1, g1, g2, g1, g3, g2,
                      s1, s2, sr, one, one, one], axis=0)
    bbl_g = np.stack([-2 * xp, -2 * yp, -2 * g1, -2 * g2, -2 * g1,
                      -2 * g3, -2 * g1, -2 * g2,
                      one, one, one, s1, s2, sr], axis=0)

    # device j-tiling: lhsT column t*P + p  <->  global j = p*T + t
    j_order = np.arange(N).reshape(P, T).T.ravel()

    def pad128(m, cols):
        out = np.zeros((128, cols), dtype=_mld.bfloat16)
        out[: m.shape[0]] = m.astype(_mld.bfloat16)
        return out

    def pad_tiles(m):
        # [nf, N] j-ordered features -> [128, T*128]: each 112-col j-tile
        # padded to 128 cols so the exp matmuls get full-width stationaries
        out = np.zeros((128, T * 128), dtype=_mld.bfloat16)
        mj = m.astype(_mld.bfloat16)
        for t in range(T):
            out[: m.shape[0], t * 128 : t * 128 + P] = \
                mj[:, t * P : (t + 1) * P]
        return out

    fbl_d = pad_tiles(abl_g[:, j_order])
    fsp_d = pad_tiles(asp_g[:, j_order])

    def hilo(v):
        hi = np.asarray(v, dtype=_mld.bfloat16).astype(np.float64)
        lo = np.asarray(v - hi, dtype=_mld.bfloat16)
        return np.stack([hi.astype(_mld.bfloat16), lo], axis=0)

    u_flat = unary.ravel()
    u_full = hilo(-4096.0 * ((1.0 - 2.0 * u_flat) - g_val))    # [2, N] bf16
    s0_pt = (1.0 - u_flat).reshape(P, T)
    s0_dev = np.zeros((P, 32), dtype=ml_dtypes.float8_e4m3fn)
    s0_dev[:, 0:14] = s0_pt[:, 0:14].astype(ml_dtypes.float8_e4m3fn)
    s0_dev[:, 16:30] = s0_pt[:, 14:28].astype(ml_dtypes.float8_e4m3fn)

    in_maps = []
    for c in range(NC):
        sl = slice(c * SHARD, (c + 1) * SHARD)
        in_maps.append({
            "fbl": fbl_d,
            "fsp": fsp_d,
            "gbl": pad128(bbl_g[:, sl], SHARD),
            "gsp": pad128(bsp_g[:, sl], SHARD),
            "u": np.ascontiguousarray(u_full[:, sl]),
            "s0": s0_dev,
            "onec": np.ones((P, 1), dtype=ml_dtypes.bfloat16),
            "one2": np.ones((2, 1), dtype=ml_dtypes.bfloat16),
            "oner": np.ones((1, P), dtype=np.float32),
        })
    return a_val, b_val, in_maps


_CACHE = {}


def kernel(inputs, spatial_ker_weights, bilateral_ker_weights,
           compatibility_matrix, _want_results=False):
    a_val, b_val, in_maps = _host_prep(
        inputs, spatial_ker_weights, bilateral_ker_weights,
        compatibility_matrix)

    key = (a_val, b_val)
    if key not in _CACHE:
        _CACHE[key] = _build(a_val, b_val)
    nc = _CACHE[key]

    res = bass_utils.run_bass_kernel_spmd(nc, in_maps, list(range(NC)))
    prob = np.concatenate([res.results[c]["out"][0] for c in range(NC)])
    out = prob.reshape(1, H, W).astype(np.float32)
    if _want_results:
        return out, nc, in_maps
    return out


if __name__ == "__main__":
    rng = np.random.default_rng(0)
    demo = {
        "inputs": rng.random((2, H, W)).astype(np.float32),
        "spatial_ker_weights":
            (rng.random((2, 2)).astype(np.float32) - 0.5) * 0.1,
        "bilateral_ker_weights":
            (rng.random((2, 2)).astype(np.float32) - 0.5) * 0.1,
        "compatibility_matrix":
            (rng.random((2, 2)).astype(np.float32) - 0.5) * 0.1,
    }
    print(kernel(**demo).shape)



# revision 58
# speedup vs baseline: 1.0678x; 1.0057x over previous
"""CRF-RNN layer (nn_CrfRnnLayer) as a Trainium2 Bass kernel on 8 NeuronCores.

Math
----
The reference iterates, for q in R^{2xN} (N=3136 pixels, 2 classes):
    s         = softmax(q, axis=0)            (or s = unaries on iter 0)
    sp_out    = (s @ K_sp) / (K_sp @ 1)
    bl_out    = (s @ K_bl) / (K_bl @ 1)
    message   = sp_w @ sp_out + bl_w @ bl_out
    q         = unaries - compat @ message
Both rows of s sum to one (softmax; unaries too), and both kernel matrices
are symmetric, so the whole update collapses to a scalar recursion on
d = q[0] - q[1]:
    s0   = sigmoid(d)                        (s = [s0, 1-s0])
    v[i] = sum_j C[j,i] * s0[j]
    d    = U - v
with C = A*K_sp/nsp + B*K_bl/nbl (column-normalized), U = (1-2u) - G, and
A, B, G scalars derived from the 2x2 weight matrices.  The final output is
softmax(q)[1] = sigmoid(-d).

Device strategy (8 cores)
-------------------------
Column-shard C: core c owns columns i in [392c, 392(c+1)).  Each core:
  1. builds its [3136 x 392] block of exp(-0.5*sqdist) for both kernels
     on-chip via PE matmuls over augmented bf16 features (hi/lo splits
     keep the exponent exact to ~2e-3), then ScalarE Exp.  The spatial
     pass absorbs ln(4096*|A|/nsp) as a feature row so its Exp lands
     directly in the prescaled fp8 C tile; the bilateral pass is column-
     summed (ones-matmuls) for 4096*B/nbl, applied by DVE before a
     halved merge-add that unblocks the first matvec groups early;
  2. iterates: a u-seed matmul plus 14 fp8 DoubleRow matmuls accumulate
     -4096*d in PSUM (s stored [112, 2, 16] so contraction pairs j-tiles
     (g, g+14) with a 16B subtile step), ScalarE sigmoid reads the PSUM
     straight into an fp8 s row, and a 392B AllGather (Shared HBM
     output) redistributes it; warm dummy matmuls keep the PE clock up
     through the ~12us collective latency, and a startup throwaway
     collective on separate buffers absorbs the CC entry barrier.
Only the last iteration skips the collective and writes sigmoid(-d).
"""

import sys

for _p in ("/root/.axon_site/_ro/trn_rl_repo", "/opt/trn_rl_repo"):
    if _p not in sys.path:
        sys.path.append(_p)

import numpy as np

import concourse.bass as bass  # noqa: F401  (registers AP types)
import concourse.tile as tile
from concourse import bacc, mybir, bass_utils

F32 = mybir.dt.float32
BF16 = mybir.dt.bfloat16
AF = mybir.ActivationFunctionType
ALU = mybir.AluOpType

H = W = 56
N = H * W            # 3136 pixels
NC = 8               # cores
SHARD = N // NC      # 392 columns per core
P = 112              # j partition-tile height (112*28 == 3136)
T = 28               # number of j tiles
ITERS = 10
TH_ALPHA, TH_BETA, TH_GAMMA = 160.0, 3.0, 3.0

_BANK = 512          # one PSUM bank, in f32 elements
_GRP = 3             # exponent tiles batched per ScalarE Exp call
_WARM = 56           # HAM warm-keeping matmuls issued under each collective.
                     # Collective latency is clock-independent but dummy
                     # duration is not: 56 ends at s-arrival on a full-clock
                     # run and overruns ~1us degraded, beating 72 in both
                     # states (72 was dummy-bound at 15.76us/18.92us periods)
_CSC = 4096.0        # fp8 prescale on C (exact power of two)
_CPAD = 392          # fp8 cmat per-tile column pitch
_GH = T // 2         # DoubleRow groups per matvec (14)


def _build(a_val: float, b_val: float) -> "bacc.Bacc":
    nc = bacc.Bacc("TRN2", target_bir_lowering=False, debug=False,
                   num_devices=NC)

    F8 = mybir.dt.float8e4
    NA = T * 128     # a-feature tiles padded 112->128 cols: full-width
                     # stationaries enable FWL (LDWEIGHTS hides behind the
                     # previous matmul's stream)
    fbl_in = nc.dram_tensor("fbl", [128, NA], BF16, kind="ExternalInput").ap()
    fsp_in = nc.dram_tensor("fsp", [128, NA], BF16, kind="ExternalInput").ap()
    gbl_in = nc.dram_tensor("gbl", [128, SHARD], BF16,
                            kind="ExternalInput").ap()
    gsp_in = nc.dram_tensor("gsp", [128, SHARD], BF16,
                            kind="ExternalInput").ap()
    u_in = nc.dram_tensor("u", [2, SHARD], BF16, kind="ExternalInput").ap()
    one2_in = nc.dram_tensor("one2", [2, 1], BF16, kind="ExternalInput").ap()
    s0_in = nc.dram_tensor("s0", [P, 32], F8, kind="ExternalInput").ap()
    onec_in = nc.dram_tensor("onec", [P, 1], BF16, kind="ExternalInput").ap()
    oner_in = nc.dram_tensor("oner", [1, P], F32, kind="ExternalInput").ap()
    out = nc.dram_tensor("out", [1, SHARD], F32, kind="ExternalOutput").ap()
    sink = nc.dram_tensor("sink", [1, 1], F32, kind="ExternalOutput").ap()
    # collective buffers: Local input, Shared output (fast HBM-HBM path).
    # The throwaway warmup collective gets its own pair so iteration 0's
    # input DMA never waits on the entry-barrier-delayed warmup read.
    di_d = [nc.dram_tensor(f"di{k}", [SHARD], F8, kind="Internal").ap()
            for k in range(3)]
    do_d = [nc.dram_tensor(f"do{k}", [N], F8, kind="Internal",
                           addr_space="Shared").ap()
            for k in range(3)]

    groups = [list(range(g, min(g + _GRP, T))) for g in range(0, T, _GRP)]

    with tile.TileContext(nc) as tc:
        with (
            tc.tile_pool(name="const", bufs=1) as cpool,
            tc.tile_pool(name="emat", bufs=1) as epool,
            tc.tile_pool(name="row", bufs=2) as rpool,
            tc.tile_pool(name="sten", bufs=2) as spool,
            tc.tile_pool(name="dram", bufs=2, space="DRAM") as dpool,
        ):
            # throwaway AllGather first, fed from a Const tensor (baked
            # into the NEFF, no runtime DMA dependency): its trigger starts
            # the wildly-variable collectives entry barrier clock as early
            # as the gpsimd queue allows
            import ml_dtypes as _ml
            dw_c = nc.inline_tensor(
                np.zeros(SHARD, dtype=_ml.float8_e4m3fn), name="dwarm").ap()
            nc.gpsimd.collective_compute(
                "AllGather", ALU.bypass,
                replica_groups=[list(range(NC))],
                ins=[dw_c[:].opt()], outs=[do_d[2][:].opt()],
            )

            # exponent-feature operands are zero-padded to 128 contraction
            # rows: a 4/5-row matmul doesn't register as PE activity, so the
            # HAM clock gate keeps the whole construction at 1.2 GHz.
            # Host sends the pad rows pre-zeroed (DMA engines are idle at
            # startup, the DVE memsets were on the critical path); the
            # bilateral operands ride first so the first exp matmul can
            # start before the spatial features land.
            # gbl (small, needed by every matmul) first; fbl arrives in
            # quarters so the first exp matmuls, which only read the
            # leading j-tiles, start ~2us before the full tensor lands
            # (the tile framework's AP-range deps make this safe)
            gbl_t = cpool.tile([128, SHARD], BF16, tag="gbl")
            nc.sync.dma_start(gbl_t[:], gbl_in[:])
            fbl_t = cpool.tile([128, NA], BF16, tag="fbl")
            for q in range(4):
                qs = slice(q * (NA // 4), (q + 1) * (NA // 4))
                nc.sync.dma_start(fbl_t[:, qs], fbl_in[:, qs])
            fsp_t = cpool.tile([128, NA], BF16, tag="fsp")
            nc.sync.dma_start(fsp_t[:], fsp_in[:])
            gsp_t = cpool.tile([128, SHARD], BF16, tag="gsp")
            nc.sync.dma_start(gsp_t[:], gsp_in[:])
            u_t = cpool.tile([2, SHARD], BF16, tag="u")
            nc.sync.dma_start(u_t[:], u_in[:])
            ones2 = cpool.tile([2, 1], BF16, tag="one2")
            nc.sync.dma_start(ones2[:], one2_in[:])
            s0_t = cpool.tile([P, 32], F8, tag="s0")
            nc.sync.dma_start(s0_t[:], s0_in[:])
            ones_col = cpool.tile([P, 1], BF16, tag="onec")
            nc.sync.dma_start(ones_col[:], onec_in[:])
            ones_row = cpool.tile([1, P], F32, tag="oner")
            nc.sync.dma_start(ones_row[:], oner_in[:])

            ebl = epool.tile([P, T * SHARD], BF16, tag="ebl")
            esp = epool.tile([P, T * SHARD], BF16, tag="esp")
            cmat = epool.tile([P, T * _CPAD], F8, tag="cmat")

            # ---- phase 1: exponent matmuls + exp + column sums ----
            # bilateral pass first: its serial tail (colsum -> recip ->
            # rb broadcast -> ebl*rb) overlaps the spatial pass, whose exp
            # (with ln(CSC*|A|/nsp) absorbed as two feature rows) lands
            # directly in the fp8 cmat.
            with (
                tc.tile_pool(name="psg", bufs=2, space="PSUM") as psg,
                tc.tile_pool(name="pss", bufs=1, space="PSUM") as pss,
            ):
                cs_bl = pss.tile([1, SHARD], F32, tag="cs_bl")

                def exp_pass(fa_t, fb_t, dst3, cs, scale, post=None):
                    for grp in groups:
                        pg = psg.tile([128, _GRP * _BANK], F32, tag="grp")
                        for k, t in enumerate(grp):
                            nc.tensor.matmul(
                                pg[:, k * _BANK : k * _BANK + SHARD],
                                fa_t[:, t * 128 : (t + 1) * 128],
                                fb_t[:],
                                start=True, stop=True,
                                skip_group_check=True,
                            )
                        ln = len(grp)
                        src = pg[:].rearrange("p (k f) -> p k f", f=_BANK)[
                            0:P, 0:ln, 0:SHARD]
                        nc.scalar.activation(dst3[:, grp[0] : grp[0] + ln, :],
                                             src, AF.Exp, scale=scale)
                        for t in (grp if cs is not None else []):
                            nc.tensor.matmul(
                                cs[:],
                                ones_col[:],
                                ebl[:, t * SHARD : (t + 1) * SHARD],
                                start=(t == 0), stop=(t == T - 1),
                                skip_group_check=True,
                            )
                        if post is not None:
                            post(grp[0], ln)

                b3 = ebl[:].rearrange("p (k f) -> p k f", f=SHARD)
                e3 = esp[:].rearrange("p (k f) -> p k f", f=SHARD)
                c3 = cmat[:].rearrange("p (k f) -> p k f", f=_CPAD
                                       )[:, :, 0:SHARD]
                exp_pass(fbl_t, gbl_t, b3, cs_bl, -1.0 / 6.0)

                # rb = CSC*B/nbl, broadcast down the partitions on the PE
                rb_row = cpool.tile([1, SHARD], F32, tag="rb")
                nc.vector.reciprocal(rb_row[:], cs_bl[:])
                rb_bc = pss.tile([P, SHARD], F32, tag="rbbc")
                nc.tensor.matmul(rb_bc[:], ones_row[:], rb_row[:],
                                 start=True, stop=True, skip_group_check=True)
                rb_sb = cpool.tile([P, SHARD], BF16, tag="rbsb")
                nc.vector.tensor_scalar_mul(rb_sb[:], rb_bc[:],
                                            float(b_val * _CSC))

                # ebl *= rb (one DVE pass) runs under the spatial Exp pass;
                # each spatial group's fp8 merge (all-bf16 inputs) then
                # chases its activation, so only the last ~0.6us of DVE work
                # trails the final Exp before the first matvec can start
                nc.vector.tensor_mul(
                    b3, b3,
                    rb_sb[:].rearrange("p (o f) -> p o f", o=1
                                       ).broadcast_to([P, T, SHARD]))

                def merge_grp(g0, ln):
                    cg = c3[:, g0 : g0 + ln, :]
                    eg = e3[:, g0 : g0 + ln, :]
                    bg = b3[:, g0 : g0 + ln, :]
                    if a_val >= 0.0:
                        nc.vector.tensor_add(cg, eg, bg)
                    else:
                        nc.vector.tensor_sub(cg, bg, eg)

                exp_pass(fsp_t, gsp_t, e3, None, 1.0 / 9.0, post=merge_grp)

                # swap the sigmoid ACT table in behind the first matvec;
                # reading the last Exp group's output pins this after the
                # Exp pass (a dep-free pre-warm gets hoisted ahead of it)
                pre_sg = cpool.tile([1, 1], F32, tag="presg")
                nc.scalar.activation(
                    pre_sg[:], esp[0:1, T * SHARD - 1 : T * SHARD],
                    AF.Sigmoid)

            # ---- phase 3: CRF mean-field iterations ----
            # psum accumulates -CSC*d = (-CSC*u seed) + sum_t CSC*C^T s.
            # fp8 DoubleRow matvec: group g contracts j-tiles (g, g+_GH);
            # s is stored [P, 2, 16] (14 used + 2 pad, 16B subtile step).
            with (
                tc.tile_pool(name="psv", bufs=2, space="PSUM") as psv,
                tc.tile_pool(name="psd", bufs=1, space="PSUM") as psd,
            ):
                dummy = psd.tile([1, SHARD], F32, tag="dummy")
                cm3 = cmat[:].rearrange("p (e g) -> p e g", e=2,
                                        g=_GH * _CPAD)
                DR = mybir.MatmulPerfMode.DoubleRow

                def seed(vt):
                    nc.tensor.matmul(
                        vt[:], ones2[:], u_t[:],
                        start=True, stop=False, skip_group_check=True,
                    )

                s_cur = s0_t
                v = psv.tile([1, SHARD], F32, tag="v")
                seed(v)
                for it in range(ITERS):
                    s3 = s_cur[:].rearrange("p (e g) -> p e g", e=2, g=16)
                    for g in range(_GH):
                        nc.tensor.matmul(
                            v[:],
                            s3[:, :, g : g + 1],
                            cm3[:, :, g * _CPAD : g * _CPAD + SHARD],
                            start=False, stop=(g == _GH - 1),
                            perf_mode=DR,
                            skip_group_check=True,
                        )
                    if it < ITERS - 1:
                        s_row = rpool.tile([1, SHARD], F8, tag="srow")
                        nc.scalar.activation(s_row[:], v[:], AF.Sigmoid,
                                             scale=-1.0 / _CSC)
                        di, do = di_d[it % 2], do_d[it % 2]
                        # scalar-issued DMA: no cross-engine semaphore hop
                        # between the sigmoid and the collective input
                        nc.scalar.dma_start(
                            di[:].rearrange("(a b) -> a b", a=1), s_row[:])
                        nc.gpsimd.collective_compute(
                            "AllGather", ALU.bypass,
                            replica_groups=[list(range(NC))],
                            ins=[di[:].opt()], outs=[do[:].opt()],
                        )
                        # keep the PE HAM-warm through the collective gap.
                        # The first ("linker") matmul reads s_row, so the
                        # whole WAW-chained dummy block is ordered after the
                        # sigmoid — it cannot interleave into the matvec
                        # accumulation and delay v's ready semaphore.  The
                        # next iteration's u-seed hides under the collective
                        # too, right behind the linker.
                        nc.tensor.matmul(
                            dummy[:], s_row[0:1, 0:1], s_row[:],
                            start=True, stop=True, skip_group_check=True,
                        )
                        v = psv.tile([1, SHARD], F32, tag="v")
                        seed(v)
                        # the last gap feeds the final matvec directly:
                        # undershoot rather than risk dummies delaying it
                        nw = _WARM if it < ITERS - 2 else _WARM - 10
                        for w in range(nw):
                            c0 = (w % T) * _CPAD
                            nc.tensor.matmul(
                                dummy[:],
                                s_cur[:, (w % 28) : (w % 28) + 1],
                                cmat[:, c0 : c0 + SHARD],
                                start=True, stop=True,
                                skip_group_check=True,
                            )
                        s_nxt = spool.tile([P, 32], F8, tag="s")
                        nc.sync.dma_start(
                            s_nxt[:].rearrange("p (e g) -> p e g",
                                               e=2, g=16)[:, :, 0:_GH],
                            do[:].rearrange("(p e g) -> p e g", e=2, g=_GH))
                        s_cur = s_nxt
                    else:
                        o_row = rpool.tile([1, SHARD], F32, tag="orow")
                        nc.scalar.activation(o_row[:], v[:], AF.Sigmoid,
                                             scale=1.0 / _CSC)
                        nc.scalar.dma_start(out[:], o_row[:])
                sink_row = rpool.tile([1, 1], F32, tag="sink")
                nc.vector.tensor_copy(sink_row[:], dummy[0:1, 0:1])
                nc.sync.dma_start(sink[:], sink_row[:])

    nc.compile()
    return nc


def _host_prep(inputs, spatial_ker_weights, bilateral_ker_weights,
               compatibility_matrix):
    unary = np.asarray(inputs[0], dtype=np.float64)
    gray = np.asarray(inputs[1], dtype=np.float64)
    sp_w = np.asarray(spatial_ker_weights, dtype=np.float64)
    bl_w = np.asarray(bilateral_ker_weights, dtype=np.float64)
    compat = np.asarray(compatibility_matrix, dtype=np.float64)

    dsp = sp_w[:, 0] - sp_w[:, 1]
    dbl = bl_w[:, 0] - bl_w[:, 1]
    c0 = sp_w[:, 1] + bl_w[:, 1]
    dc = compat[0, :] - compat[1, :]
    a_val = float(dc @ dsp)
    b_val = float(dc @ dbl)
    g_val = float(dc @ c0)

    ys, xs = np.meshgrid(np.arange(H, dtype=np.float64),
                         np.arange(W, dtype=np.float64), indexing="ij")
    x = xs.ravel()
    y = ys.ravel()
    gf = gray.ravel() * 255.0

    import ml_dtypes
    _mld = ml_dtypes
    one = np.ones(N, dtype=np.float64)

    def bf(v):
        return np.asarray(v, dtype=_mld.bfloat16).astype(np.float64)

    def split3(v):
        a = bf(v)
        b = bf(v - a)
        c = bf(v - a - b)
        return a, b, c

    # spatial norm is a Kronecker product: nsp[(y,x)] = ry[y]*rx[x]
    idx = np.arange(H, dtype=np.float64)
    g1d = np.exp(-0.5 * ((idx[None, :] - idx[:, None]) / TH_GAMMA) ** 2)
    r1d = g1d.sum(axis=1)
    nsp = (r1d[y.astype(int)] * r1d[x.astype(int)])

    # spatial exponent in bf16-exact integer arithmetic, scaled by 1/9 at
    # the Exp activation; the fp8 prescale and the column norm are folded
    # in as a 9*ln(CSC*|A|/nsp) hi/lo feature pair:
    # presc = xj*xi + yj*yi - (x^2+y^2)/2 terms + ln rows
    ssp_i = 0.5 * (x * x + y * y)                 # multiples of 0.5
    sp_hi = bf(-ssp_i)
    sp_lo = -ssp_i - sp_hi                        # exact in bf16
    lr = 9.0 * np.log(np.maximum(4096.0 * abs(a_val) / nsp, 1e-280))
    lr = np.maximum(lr, -2000.0)
    lr_hi = bf(lr)
    lr_lo = lr - lr_hi

    asp_g = np.stack([x, y, one, one, sp_hi, sp_lo, one, one], axis=0)
    bsp_g = np.stack([x, y, sp_hi, sp_lo, one, one, lr_hi, lr_lo], axis=0)

    # bilateral exponent on bf16 features: presc = ssq_i + ssq_j
    # - 2*(xp_i xp_j + yp_i yp_j + g_i g_j), scaled by -1/6 at the Exp.
    # g and ssq are 3-way bf16 splits so every product is exact in f32;
    # validated max exponent error ~2e-3.
    s3f = np.sqrt(3.0)
    xp = bf(s3f * x / TH_ALPHA)
    yp = bf(s3f * y / TH_ALPHA)
    g1, g2, g3 = split3(gf)
    gs = g1 + g2 + g3
    ssq = xp * xp + yp * yp + gs * gs
    s1, s2, sr = split3(ssq)

    abl_g = np.stack([xp, yp, g1, g1, g2, g1, g3, g2,
                      s1, s2, sr, one, one, one], axis=0)
    bbl_g = np.stack([-2 * xp, -2 * yp, -2 * g1, -2 * g2, -2 * g1,
                      -2 * g3, -2 * g1, -2 * g2,
                      one, one, one, s1, s2, sr], axis=0)

    # device j-tiling: lhsT column t*P + p  <->  global j = p*T + t
    j_order = np.arange(N).reshape(P, T).T.ravel()

    def pad128(m, cols):
        out = np.zeros((128, cols), dtype=_mld.bfloat16)
        out[: m.shape[0]] = m.astype(_mld.bfloat16)
        return out

    def pad_tiles(m):
        # [nf, N] j-ordered features -> [128, T*128]: each 112-col j-tile
        # padded to 128 cols so the exp matmuls get full-width stationaries
        out = np.zeros((128, T * 128), dtype=_mld.bfloat16)
        mj = m.astype(_mld.bfloat16)
        for t in range(T):
            out[: m.shape[0], t * 128 : t * 128 + P] = \
                mj[:, t * P : (t + 1) * P]
        return out

    fbl_d = pad_tiles(abl_g[:, j_order])
    fsp_d = pad_tiles(asp_g[:, j_order])

    def hilo(v):
        hi = np.asarray(v, dtype=_mld.bfloat16).astype(np.float64)
        lo = np.asarray(v - hi, dtype=_mld.bfloat16)
        return np.stack([hi.astype(_mld.bfloat16), lo], axis=0)

    u_flat = unary.ravel()
    u_full = hilo(-4096.0 * ((1.0 - 2.0 * u_flat) - g_val))    # [2, N] bf16
    s0_pt = (1.0 - u_flat).reshape(P, T)
    s0_dev = np.zeros((P, 32), dtype=ml_dtypes.float8_e4m3fn)
    s0_dev[:, 0:14] = s0_pt[:, 0:14].astype(ml_dtypes.float8_e4m3fn)
    s0_dev[:, 16:30] = s0_pt[:, 14:28].astype(ml_dtypes.float8_e4m3fn)

    in_maps = []
    for c in range(NC):
        sl = slice(c * SHARD, (c + 1) * SHARD)
        in_maps.append({
            "fbl": fbl_d,
            "fsp": fsp_d,
            "gbl": pad128(bbl_g[:, sl], SHARD),
            "gsp": pad128(bsp_g[:, sl], SHARD),
            "u": np.ascontiguousarray(u_full[:, sl]),
            "s0": s0_dev,
            "onec": np.ones((P, 1), dtype=ml_dtypes.bfloat16),
            "one2": np.ones((2, 1), dtype=ml_dtypes.bfloat16),
            "oner": np.ones((1, P), dtype=np.float32),
        })
    return a_val, b_val, in_maps


_CACHE = {}


def kernel(inputs, spatial_ker_weights, bilateral_ker_weights,
           compatibility_matrix, _want_results=False):
    a_val, b_val, in_maps = _host_prep(
        inputs, spatial_ker_weights, bilateral_ker_weights,
        compatibility_matrix)

    key = (a_val, b_val)
    if key not in _CACHE:
        _CACHE[key] = _build(a_val, b_val)
    nc = _CACHE[key]

    res = bass_utils.run_bass_kernel_spmd(nc, in_maps, list(range(NC)))
    prob = np.concatenate([res.results[c]["out"][0] for c in range(NC)])
    out = prob.reshape(1, H, W).astype(np.float32)
    if _want_results:
        return out, nc, in_maps
    return out


if __name__ == "__main__":
    rng = np.random.default_rng(0)
    demo = {
        "inputs": rng.random((2, H, W)).astype(np.float32),
        "spatial_ker_weights":
            (rng.random((2, 2)).astype(np.float32) - 0.5) * 0.1,
        "bilateral_ker_weights":
            (rng.random((2, 2)).astype(np.float32) - 0.5) * 0.1,
        "compatibility_matrix":
            (rng.random((2, 2)).astype(np.float32) - 0.5) * 0.1,
    }
    print(kernel(**demo).shape)



# revision 59
# speedup vs baseline: 1.0891x; 1.0199x over previous
"""CRF-RNN layer (nn_CrfRnnLayer) as a Trainium2 Bass kernel on 8 NeuronCores.

Math
----
The reference iterates, for q in R^{2xN} (N=3136 pixels, 2 classes):
    s         = softmax(q, axis=0)            (or s = unaries on iter 0)
    sp_out    = (s @ K_sp) / (K_sp @ 1)
    bl_out    = (s @ K_bl) / (K_bl @ 1)
    message   = sp_w @ sp_out + bl_w @ bl_out
    q         = unaries - compat @ message
Both rows of s sum to one (softmax; unaries too), and both kernel matrices
are symmetric, so the whole update collapses to a scalar recursion on
d = q[0] - q[1]:
    s0   = sigmoid(d)                        (s = [s0, 1-s0])
    v[i] = sum_j C[j,i] * s0[j]
    d    = U - v
with C = A*K_sp/nsp + B*K_bl/nbl (column-normalized), U = (1-2u) - G, and
A, B, G scalars derived from the 2x2 weight matrices.  The final output is
softmax(q)[1] = sigmoid(-d).

Device strategy (8 cores)
-------------------------
Column-shard C: core c owns columns i in [392c, 392(c+1)).  Each core:
  1. builds its [3136 x 392] block of exp(-0.5*sqdist) for both kernels
     on-chip via PE matmuls over augmented bf16 features (hi/lo splits
     keep the exponent exact to ~2e-3), then ScalarE Exp.  The spatial
     pass absorbs ln(4096*|A|/nsp) as a feature row so its Exp lands
     directly in the prescaled fp8 C tile; the bilateral pass is column-
     summed (ones-matmuls) for 4096*B/nbl, applied by DVE before a
     halved merge-add that unblocks the first matvec groups early;
  2. iterates: a u-seed matmul plus 14 fp8 DoubleRow matmuls accumulate
     -4096*d in PSUM (s stored [112, 2, 16] so contraction pairs j-tiles
     (g, g+14) with a 16B subtile step), ScalarE sigmoid reads the PSUM
     straight into an fp8 s row, and a 392B AllGather (Shared HBM
     output) redistributes it; warm dummy matmuls keep the PE clock up
     through the ~12us collective latency, and a startup throwaway
     collective on separate buffers absorbs the CC entry barrier.
Only the last iteration skips the collective and writes sigmoid(-d).
"""

import sys

for _p in ("/root/.axon_site/_ro/trn_rl_repo", "/opt/trn_rl_repo"):
    if _p not in sys.path:
        sys.path.append(_p)

import numpy as np

import concourse.bass as bass  # noqa: F401  (registers AP types)
import concourse.tile as tile
from concourse import bacc, mybir, bass_utils

F32 = mybir.dt.float32
BF16 = mybir.dt.bfloat16
AF = mybir.ActivationFunctionType
ALU = mybir.AluOpType

H = W = 56
N = H * W            # 3136 pixels
NC = 8               # cores
SHARD = N // NC      # 392 columns per core
P = 112              # j partition-tile height (112*28 == 3136)
T = 28               # number of j tiles
ITERS = 10
TH_ALPHA, TH_BETA, TH_GAMMA = 160.0, 3.0, 3.0

_BANK = 512          # one PSUM bank, in f32 elements
_GRP = 3             # exponent tiles batched per ScalarE Exp call
_WARM = 56           # HAM warm-keeping matmuls issued under each collective.
                     # Collective latency is clock-independent but dummy
                     # duration is not: 56 ends at s-arrival on a full-clock
                     # run and overruns ~1us degraded, beating 72 in both
                     # states (72 was dummy-bound at 15.76us/18.92us periods)
_CSC = 4096.0        # fp8 prescale on C (exact power of two)
_CPAD = 392          # fp8 cmat per-tile column pitch
_GH = T // 2         # DoubleRow groups per matvec (14)


def _build(a_val: float, b_val: float) -> "bacc.Bacc":
    nc = bacc.Bacc("TRN2", target_bir_lowering=False, debug=False,
                   num_devices=NC)

    F8 = mybir.dt.float8e4
    NA = T * 128     # a-feature tiles padded 112->128 cols: full-width
                     # stationaries enable FWL (LDWEIGHTS hides behind the
                     # previous matmul's stream)
    fbl_in = nc.dram_tensor("fbl", [128, NA], BF16, kind="ExternalInput").ap()
    fsp_in = nc.dram_tensor("fsp", [128, NA], BF16, kind="ExternalInput").ap()
    gbl_in = nc.dram_tensor("gbl", [128, SHARD], BF16,
                            kind="ExternalInput").ap()
    gsp_in = nc.dram_tensor("gsp", [128, SHARD], BF16,
                            kind="ExternalInput").ap()
    u_in = nc.dram_tensor("u", [2, SHARD], BF16, kind="ExternalInput").ap()
    one2_in = nc.dram_tensor("one2", [2, 1], BF16, kind="ExternalInput").ap()
    s0_in = nc.dram_tensor("s0", [P, 32], F8, kind="ExternalInput").ap()
    onec_in = nc.dram_tensor("onec", [P, 1], BF16, kind="ExternalInput").ap()
    oner_in = nc.dram_tensor("oner", [1, P], F32, kind="ExternalInput").ap()
    out = nc.dram_tensor("out", [1, SHARD], F32, kind="ExternalOutput").ap()
    sink = nc.dram_tensor("sink", [1, 1], F32, kind="ExternalOutput").ap()
    # collective buffers: Local input, Shared output (fast HBM-HBM path).
    # The throwaway warmup collective gets its own pair so iteration 0's
    # input DMA never waits on the entry-barrier-delayed warmup read.
    di_d = [nc.dram_tensor(f"di{k}", [SHARD], F8, kind="Internal").ap()
            for k in range(3)]
    do_d = [nc.dram_tensor(f"do{k}", [N], F8, kind="Internal",
                           addr_space="Shared").ap()
            for k in range(3)]

    groups = [list(range(g, min(g + _GRP, T))) for g in range(0, T, _GRP)]

    with tile.TileContext(nc) as tc:
        with (
            tc.tile_pool(name="const", bufs=1) as cpool,
            tc.tile_pool(name="emat", bufs=1) as epool,
            tc.tile_pool(name="row", bufs=2) as rpool,
            tc.tile_pool(name="sten", bufs=2) as spool,
            tc.tile_pool(name="dram", bufs=2, space="DRAM") as dpool,
        ):
            # throwaway AllGather first, fed from a Const tensor (baked
            # into the NEFF, no runtime DMA dependency): its trigger starts
            # the wildly-variable collectives entry barrier clock as early
            # as the gpsimd queue allows
            import ml_dtypes as _ml
            dw_c = nc.inline_tensor(
                np.zeros(SHARD, dtype=_ml.float8_e4m3fn), name="dwarm").ap()
            nc.gpsimd.collective_compute(
                "AllGather", ALU.bypass,
                replica_groups=[list(range(NC))],
                ins=[dw_c[:].opt()], outs=[do_d[2][:].opt()],
            )

            # exponent-feature operands are zero-padded to 128 contraction
            # rows: a 4/5-row matmul doesn't register as PE activity, so the
            # HAM clock gate keeps the whole construction at 1.2 GHz.
            # Host sends the pad rows pre-zeroed (DMA engines are idle at
            # startup, the DVE memsets were on the critical path); the
            # bilateral operands ride first so the first exp matmul can
            # start before the spatial features land.
            # gbl (small, needed by every matmul) first; fbl arrives in
            # quarters so the first exp matmuls, which only read the
            # leading j-tiles, start ~2us before the full tensor lands
            # (the tile framework's AP-range deps make this safe)
            gbl_t = cpool.tile([128, SHARD], BF16, tag="gbl")
            nc.sync.dma_start(gbl_t[:], gbl_in[:])
            fbl_t = cpool.tile([128, NA], BF16, tag="fbl")
            for q in range(4):
                qs = slice(q * (NA // 4), (q + 1) * (NA // 4))
                nc.sync.dma_start(fbl_t[:, qs], fbl_in[:, qs])
            fsp_t = cpool.tile([128, NA], BF16, tag="fsp")
            nc.sync.dma_start(fsp_t[:], fsp_in[:])
            gsp_t = cpool.tile([128, SHARD], BF16, tag="gsp")
            nc.sync.dma_start(gsp_t[:], gsp_in[:])
            u_t = cpool.tile([2, SHARD], BF16, tag="u")
            nc.sync.dma_start(u_t[:], u_in[:])
            ones2 = cpool.tile([2, 1], BF16, tag="one2")
            nc.sync.dma_start(ones2[:], one2_in[:])
            s0_t = cpool.tile([P, 32], F8, tag="s0")
            nc.sync.dma_start(s0_t[:], s0_in[:])
            ones_col = cpool.tile([P, 1], BF16, tag="onec")
            nc.sync.dma_start(ones_col[:], onec_in[:])
            ones_row = cpool.tile([1, P], F32, tag="oner")
            nc.sync.dma_start(ones_row[:], oner_in[:])

            ebl = epool.tile([P, T * SHARD], BF16, tag="ebl")
            esp = epool.tile([P, T * SHARD], BF16, tag="esp")
            cmat = epool.tile([P, T * _CPAD], F8, tag="cmat")

            # ---- phase 1: exponent matmuls + exp + column sums ----
            # bilateral pass first: its serial tail (colsum -> recip ->
            # rb broadcast -> ebl*rb) overlaps the spatial pass, whose exp
            # (with ln(CSC*|A|/nsp) absorbed as two feature rows) lands
            # directly in the fp8 cmat.
            with (
                tc.tile_pool(name="psg", bufs=2, space="PSUM") as psg,
                tc.tile_pool(name="pss", bufs=1, space="PSUM") as pss,
            ):
                cs_bl = pss.tile([1, SHARD], F32, tag="cs_bl")

                def exp_pass(fa_t, fb_t, dst3, cs, scale, post=None):
                    for grp in groups:
                        pg = psg.tile([128, _GRP * _BANK], F32, tag="grp")
                        for k, t in enumerate(grp):
                            nc.tensor.matmul(
                                pg[:, k * _BANK : k * _BANK + SHARD],
                                fa_t[:, t * 128 : (t + 1) * 128],
                                fb_t[:],
                                start=True, stop=True,
                                skip_group_check=True,
                            )
                        ln = len(grp)
                        src = pg[:].rearrange("p (k f) -> p k f", f=_BANK)[
                            0:P, 0:ln, 0:SHARD]
                        nc.scalar.activation(dst3[:, grp[0] : grp[0] + ln, :],
                                             src, AF.Exp, scale=scale)
                        for t in (grp if cs is not None else []):
                            nc.tensor.matmul(
                                cs[:],
                                ones_col[:],
                                ebl[:, t * SHARD : (t + 1) * SHARD],
                                start=(t == 0), stop=(t == T - 1),
                                skip_group_check=True,
                            )
                        if post is not None:
                            post(grp[0], ln)

                b3 = ebl[:].rearrange("p (k f) -> p k f", f=SHARD)
                e3 = esp[:].rearrange("p (k f) -> p k f", f=SHARD)
                c3 = cmat[:].rearrange("p (k f) -> p k f", f=_CPAD
                                       )[:, :, 0:SHARD]
                exp_pass(fbl_t, gbl_t, b3, cs_bl, -1.0 / 6.0)

                # rb = CSC*B/nbl, broadcast down the partitions on the PE
                rb_row = cpool.tile([1, SHARD], F32, tag="rb")
                nc.vector.reciprocal(rb_row[:], cs_bl[:])
                rb_bc = pss.tile([P, SHARD], F32, tag="rbbc")
                nc.tensor.matmul(rb_bc[:], ones_row[:], rb_row[:],
                                 start=True, stop=True, skip_group_check=True)
                rb_sb = cpool.tile([P, SHARD], BF16, tag="rbsb")
                nc.vector.tensor_scalar_mul(rb_sb[:], rb_bc[:],
                                            float(b_val * _CSC))

                # ebl *= rb (one DVE pass) runs under the spatial Exp pass;
                # each spatial group's fp8 merge (all-bf16 inputs) then
                # chases its activation, so only the last ~0.6us of DVE work
                # trails the final Exp before the first matvec can start
                nc.vector.tensor_mul(
                    b3, b3,
                    rb_sb[:].rearrange("p (o f) -> p o f", o=1
                                       ).broadcast_to([P, T, SHARD]))

                def merge_grp(g0, ln):
                    cg = c3[:, g0 : g0 + ln, :]
                    eg = e3[:, g0 : g0 + ln, :]
                    bg = b3[:, g0 : g0 + ln, :]
                    if a_val >= 0.0:
                        nc.vector.tensor_add(cg, eg, bg)
                    else:
                        nc.vector.tensor_sub(cg, bg, eg)

                exp_pass(fsp_t, gsp_t, e3, None, 1.0 / 9.0, post=merge_grp)

                # swap the sigmoid ACT table in behind the first matvec;
                # reading the last Exp group's output pins this after the
                # Exp pass (a dep-free pre-warm gets hoisted ahead of it)
                pre_sg = cpool.tile([1, 1], F32, tag="presg")
                nc.scalar.activation(
                    pre_sg[:], esp[0:1, T * SHARD - 1 : T * SHARD],
                    AF.Sigmoid)

            # ---- phase 3: CRF mean-field iterations ----
            # psum accumulates -CSC*d = (-CSC*u seed) + sum_t CSC*C^T s.
            # fp8 DoubleRow matvec: group g contracts j-tiles (g, g+_GH);
            # s is stored [P, 2, 16] (14 used + 2 pad, 16B subtile step).
            with (
                tc.tile_pool(name="psv", bufs=2, space="PSUM") as psv,
                tc.tile_pool(name="psd", bufs=1, space="PSUM") as psd,
            ):
                dummy = psd.tile([1, SHARD], F32, tag="dummy")
                cm3 = cmat[:].rearrange("p (e g) -> p e g", e=2,
                                        g=_GH * _CPAD)
                DR = mybir.MatmulPerfMode.DoubleRow

                def seed(vt):
                    nc.tensor.matmul(
                        vt[:], ones2[:], u_t[:],
                        start=True, stop=False, skip_group_check=True,
                    )

                s_cur = s0_t
                v = psv.tile([1, SHARD], F32, tag="v")
                seed(v)
                for it in range(ITERS):
                    s3 = s_cur[:].rearrange("p (e g) -> p e g", e=2, g=16)
                    for g in range(_GH):
                        nc.tensor.matmul(
                            v[:],
                            s3[:, :, g : g + 1],
                            cm3[:, :, g * _CPAD : g * _CPAD + SHARD],
                            start=False, stop=(g == _GH - 1),
                            perf_mode=DR,
                            skip_group_check=True,
                        )
                    if it < ITERS - 1:
                        s_row = rpool.tile([1, SHARD], F8, tag="srow")
                        nc.scalar.activation(s_row[:], v[:], AF.Sigmoid,
                                             scale=-1.0 / _CSC)
                        di, do = di_d[it % 2], do_d[it % 2]
                        # scalar-issued DMA: no cross-engine semaphore hop
                        # between the sigmoid and the collective input
                        nc.scalar.dma_start(
                            di[:].rearrange("(a b) -> a b", a=1), s_row[:])
                        nc.gpsimd.collective_compute(
                            "AllGather", ALU.bypass,
                            replica_groups=[list(range(NC))],
                            ins=[di[:].opt()], outs=[do[:].opt()],
                        )
                        # keep the PE HAM-warm through the collective gap.
                        # The first ("linker") matmul reads s_row, so the
                        # whole WAW-chained dummy block is ordered after the
                        # sigmoid — it cannot interleave into the matvec
                        # accumulation and delay v's ready semaphore.  The
                        # next iteration's u-seed hides under the collective
                        # too, right behind the linker.
                        nc.tensor.matmul(
                            dummy[:], s_row[0:1, 0:1], s_row[:],
                            start=True, stop=True, skip_group_check=True,
                        )
                        v = psv.tile([1, SHARD], F32, tag="v")
                        seed(v)
                        # the final collective's mesh runs consistently
                        # slower (7.4-8.9us vs 5.5-6.5 steady), so the full
                        # block still undershoots s-arrival there
                        for w in range(_WARM):
                            c0 = (w % T) * _CPAD
                            nc.tensor.matmul(
                                dummy[:],
                                s_cur[:, (w % 28) : (w % 28) + 1],
                                cmat[:, c0 : c0 + SHARD],
                                start=True, stop=True,
                                skip_group_check=True,
                            )
                        s_nxt = spool.tile([P, 32], F8, tag="s")
                        nc.sync.dma_start(
                            s_nxt[:].rearrange("p (e g) -> p e g",
                                               e=2, g=16)[:, :, 0:_GH],
                            do[:].rearrange("(p e g) -> p e g", e=2, g=_GH))
                        s_cur = s_nxt
                    else:
                        o_row = rpool.tile([1, SHARD], F32, tag="orow")
                        nc.scalar.activation(o_row[:], v[:], AF.Sigmoid,
                                             scale=1.0 / _CSC)
                        nc.scalar.dma_start(out[:], o_row[:])
                sink_row = rpool.tile([1, 1], F32, tag="sink")
                nc.vector.tensor_copy(sink_row[:], dummy[0:1, 0:1])
                nc.sync.dma_start(sink[:], sink_row[:])

    nc.compile()
    return nc


def _host_prep(inputs, spatial_ker_weights, bilateral_ker_weights,
               compatibility_matrix):
    unary = np.asarray(inputs[0], dtype=np.float64)
    gray = np.asarray(inputs[1], dtype=np.float64)
    sp_w = np.asarray(spatial_ker_weights, dtype=np.float64)
    bl_w = np.asarray(bilateral_ker_weights, dtype=np.float64)
    compat = np.asarray(compatibility_matrix, dtype=np.float64)

    dsp = sp_w[:, 0] - sp_w[:, 1]
    dbl = bl_w[:, 0] - bl_w[:, 1]
    c0 = sp_w[:, 1] + bl_w[:, 1]
    dc = compat[0, :] - compat[1, :]
    a_val = float(dc @ dsp)
    b_val = float(dc @ dbl)
    g_val = float(dc @ c0)

    ys, xs = np.meshgrid(np.arange(H, dtype=np.float64),
                         np.arange(W, dtype=np.float64), indexing="ij")
    x = xs.ravel()
    y = ys.ravel()
    gf = gray.ravel() * 255.0

    import ml_dtypes
    _mld = ml_dtypes
    one = np.ones(N, dtype=np.float64)

    def bf(v):
        return np.asarray(v, dtype=_mld.bfloat16).astype(np.float64)

    def split3(v):
        a = bf(v)
        b = bf(v - a)
        c = bf(v - a - b)
        return a, b, c

    # spatial norm is a Kronecker product: nsp[(y,x)] = ry[y]*rx[x]
    idx = np.arange(H, dtype=np.float64)
    g1d = np.exp(-0.5 * ((idx[None, :] - idx[:, None]) / TH_GAMMA) ** 2)
    r1d = g1d.sum(axis=1)
    nsp = (r1d[y.astype(int)] * r1d[x.astype(int)])

    # spatial exponent in bf16-exact integer arithmetic, scaled by 1/9 at
    # the Exp activation; the fp8 prescale and the column norm are folded
    # in as a 9*ln(CSC*|A|/nsp) hi/lo feature pair:
    # presc = xj*xi + yj*yi - (x^2+y^2)/2 terms + ln rows
    ssp_i = 0.5 * (x * x + y * y)                 # multiples of 0.5
    sp_hi = bf(-ssp_i)
    sp_lo = -ssp_i - sp_hi                        # exact in bf16
    lr = 9.0 * np.log(np.maximum(4096.0 * abs(a_val) / nsp, 1e-280))
    lr = np.maximum(lr, -2000.0)
    lr_hi = bf(lr)
    lr_lo = lr - lr_hi

    asp_g = np.stack([x, y, one, one, sp_hi, sp_lo, one, one], axis=0)
    bsp_g = np.stack([x, y, sp_hi, sp_lo, one, one, lr_hi, lr_lo], axis=0)

    # bilateral exponent on bf16 features: presc = ssq_i + ssq_j
    # - 2*(xp_i xp_j + yp_i yp_j + g_i g_j), scaled by -1/6 at the Exp.
    # g and ssq are 3-way bf16 splits so every product is exact in f32;
    # validated max exponent error ~2e-3.
    s3f = np.sqrt(3.0)
    xp = bf(s3f * x / TH_ALPHA)
    yp = bf(s3f * y / TH_ALPHA)
    g1, g2, g3 = split3(gf)
    gs = g1 + g2 + g3
    ssq = xp * xp + yp * yp + gs * gs
    s1, s2, sr = split3(ssq)

    abl_g = np.stack([xp, yp, g1, g1, g2, g1, g3, g2,
                      s1, s2, sr, one, one, one], axis=0)
    bbl_g = np.stack([-2 * xp, -2 * yp, -2 * g1, -2 * g2, -2 * g1,
                      -2 * g3, -2 * g1, -2 * g2,
                      one, one, one, s1, s2, sr], axis=0)

    # device j-tiling: lhsT column t*P + p  <->  global j = p*T + t
    j_order = np.arange(N).reshape(P, T).T.ravel()

    def pad128(m, cols):
        out = np.zeros((128, cols), dtype=_mld.bfloat16)
        out[: m.shape[0]] = m.astype(_mld.bfloat16)
        return out

    def pad_tiles(m):
        # [nf, N] j-ordered features -> [128, T*128]: each 112-col j-tile
        # padded to 128 cols so the exp matmuls get full-width stationaries
        out = np.zeros((128, T * 128), dtype=_mld.bfloat16)
        mj = m.astype(_mld.bfloat16)
        for t in range(T):
            out[: m.shape[0], t * 128 : t * 128 + P] = \
                mj[:, t * P : (t + 1) * P]
        return out

    fbl_d = pad_tiles(abl_g[:, j_order])
    fsp_d = pad_tiles(asp_g[:, j_order])

    def hilo(v):
        hi = np.asarray(v, dtype=_mld.bfloat16).astype(np.float64)
        lo = np.asarray(v - hi, dtype=_mld.bfloat16)
        return np.stack([hi.astype(_mld.bfloat16), lo], axis=0)

    u_flat = unary.ravel()
    u_full = hilo(-4096.0 * ((1.0 - 2.0 * u_flat) - g_val))    # [2, N] bf16
    s0_pt = (1.0 - u_flat).reshape(P, T)
    s0_dev = np.zeros((P, 32), dtype=ml_dtypes.float8_e4m3fn)
    s0_dev[:, 0:14] = s0_pt[:, 0:14].astype(ml_dtypes.float8_e4m3fn)
    s0_dev[:, 16:30] = s0_pt[:, 14:28].astype(ml_dtypes.float8_e4m3fn)

    in_maps = []
    for c in range(NC):
        sl = slice(c * SHARD, (c + 1) * SHARD)
        in_maps.append({
            "fbl": fbl_d,
            "fsp": fsp_d,
            "gbl": pad128(bbl_g[:, sl], SHARD),
            "gsp": pad128(bsp_g[:, sl], SHARD),
            "u": np.ascontiguousarray(u_full[:, sl]),
            "s0": s0_dev,
            "onec": np.ones((P, 1), dtype=ml_dtypes.bfloat16),
            "one2": np.ones((2, 1), dtype=ml_dtypes.bfloat16),
            "oner": np.ones((1, P), dtype=np.float32),
        })
    return a_val, b_val, in_maps


_CACHE = {}


def kernel(inputs, spatial_ker_weights, bilateral_ker_weights,
           compatibility_matrix, _want_results=False):
    a_val, b_val, in_maps = _host_prep(
        inputs, spatial_ker_weights, bilateral_ker_weights,
        compatibility_matrix)

    key = (a_val, b_val)
    if key not in _CACHE:
        _CACHE[key] = _build(a_val, b_val)
    nc = _CACHE[key]

    res = bass_utils.run_bass_kernel_spmd(nc, in_maps, list(range(NC)))
    prob = np.concatenate([res.results[c]["out"][0] for c in range(NC)])
    out = prob.reshape(1, H, W).astype(np.float32)
    if _want_results:
        return out, nc, in_maps
    return out


if __name__ == "__main__":
    rng = np.random.default_rng(0)
    demo = {
        "inputs": rng.random((2, H, W)).astype(np.float32),
        "spatial_ker_weights":
            (rng.random((2, 2)).astype(np.float32) - 0.5) * 0.1,
        "bilateral_ker_weights":
            (rng.random((2, 2)).astype(np.float32) - 0.5) * 0.1,
        "compatibility_matrix":
            (rng.random((2, 2)).astype(np.float32) - 0.5) * 0.1,
    }
    print(kernel(**demo).shape)

